# revision 16
# baseline (speedup 1.0000x reference)
"""Trainium2 Bass kernel for nn_DecoderLayer (gnn_message_passing).

Sharding: flatten B*N = 4096 nodes, 512 nodes per core across 8 cores.

Math per node n, neighbor k (reference):
  h_EV = [h_V[n], h_E[n,k]]                                (128+384)
  h1 = gelu(h_EV @ W1.T + b1); h2 = gelu(h1 @ W2.T + b2)
  msg = h2 @ W3.T + b3
  dh  = sum_k mask_attend[n,k] * msg / 30
  h   = LN1(h_V + dh)
  dh2 = gelu(h @ Win.T + bin) @ Wout.T + bout
  out = mask_V[n] * LN2(h + dh2)

Key folds:
  - h_E streamed as fp8-e4m3 (4x less HBM than f32; rel-err budget is 2e-2
    and fp8 h_E costs ~1.3e-3 end to end). Plain 3-chunk fp8 matmuls
    (DoubleRow measured no faster on HW - it streams 2N columns).
  - W1 split: W1 = [W1V | W1E]; hv1 = W1V^T h_V computed once per node and
    K-replicated into PSUM via a bf16 identity matmul with a stride-0 AP.
  - W2 lagged one group behind W1 in the PE program so the PE never waits
    on gelu1 -> no PE idle gaps -> HAM stays at 2.4 GHz.
  - masked K-sum moved before W3: m2 = sum_k mask*h2 (DVE pairwise tree at
    bf16 2x rate); dh^T = (m2 block as stationary) @ W3^T so the result
    lands already transposed for LN - no PE transpose / copy for dh.
  - dh2^T likewise: ffr block as stationary, Wout^T moving.
  - per-block tail chains (dh, LN1, FFN, LN2, store) interleave with the
    tail of the main loop using the 2 spare PSUM banks.
  - LN rstd via bit-hack rsqrt + Newton on DVE: the ACT engine never loads
    the Sqrt table, so the Gelu table persists (saves 4 table swaps).
"""

from contextlib import ExitStack

import numpy as np

import concourse.bacc as bacc
import concourse.tile as tile
from concourse import mybir
from concourse.bass_utils import run_bass_kernel_spmd

F32 = mybir.dt.float32
F32R = mybir.dt.float32r
BF16 = mybir.dt.bfloat16
F8 = mybir.dt.float8e4
U32 = mybir.dt.uint32
AF = mybir.ActivationFunctionType
ALU = mybir.AluOpType
AX = mybir.AxisListType

H = 128
NIN = 384
NCHUNK = NIN // 128  # 3
FF = 512
FCHUNK = FF // 128   # 4
K = 48
SCALE = 30.0
EPS = 1e-5
NCORES = 8

TT = 384                 # rows per matmul tile (8 nodes * 48)
NPT = TT // K            # 8 nodes per tile
QT = 2                   # tiles per group (2 PSUM banks)
GR = TT * QT             # 768 rows per group

GELU = AF.Gelu  # swapped for sim (CoreSim lacks Gelu)

# packed f32 const layout (columns)
_PK32 = {"b1": (0, 1), "b2": (1, 1), "b3rep": (2, 128), "binp": (130, 4),
         "boutrep": (134, 128), "g1rep": (262, 128), "b1rep": (390, 128),
         "g2rep": (518, 128), "b2rep": (646, 128), "identf": (774, 128),
         "epsv": (902, 1)}
PK32_COLS = 903
# packed f32r const layout
_PKR = {"w1vt": (0, 128), "wint": (128, FF), "woutt": (640, FCHUNK * 128)}
PKR_COLS = 1152
# packed bf16 const layout
_PKB = {"w2t": (0, 128), "w3t": (128, 128), "identb": (256, 128)}
PKB_COLS = 384
# packed fp8 const layout: w1et chunks [128, 3*128]
PK8_COLS = NCHUNK * 128

MAGIC = 0x5F3759DF


def _emit(nc, io, npc):
    rows = npc * K
    ngrp = rows // GR
    nblk = npc // 128
    gpb = ngrp // nblk  # groups per 128-node block (8)
    assert rows % GR == 0 and npc % 128 == 0

    with tile.TileContext(nc) as tc, ExitStack() as ctx:
        cpool = ctx.enter_context(tc.tile_pool(name="const", bufs=1))
        small = ctx.enter_context(tc.tile_pool(name="small", bufs=4))
        hpool = ctx.enter_context(tc.tile_pool(name="he", bufs=3))
        wpool = ctx.enter_context(tc.tile_pool(name="work", bufs=2))
        lnpool = ctx.enter_context(tc.tile_pool(name="ln", bufs=2))

        # prefetch the first he quad ahead of everything on the sync queue;
        # group 0's slice goes first so W1(0) can start ~1us in
        h_et_d = io["h_et"][:]
        he4_0 = hpool.tile([128, 4, NCHUNK * GR], F8, tag="he")
        nc.sync.dma_start(he4_0[:, 0, :], h_et_d[0][:, 0:NCHUNK * GR])
        nc.sync.dma_start(he4_0[:, 1:4, :],
                          h_et_d[0][:, NCHUNK * GR:4 * NCHUNK * GR])

        # ---- packed constants (few big DMAs) ----
        pk32 = cpool.tile([128, PK32_COLS], F32, tag="pk32")
        nc.sync.dma_start(pk32[:], io["pk32"][:])
        pkr = cpool.tile([128, PKR_COLS], F32R, tag="pkr")
        nc.sync.dma_start(pkr[:], io["pkr"][:])
        pkb = cpool.tile([128, PKB_COLS], BF16, tag="pkb")
        nc.sync.dma_start(pkb[:], io["pkb"][:])
        pk8 = cpool.tile([128, PK8_COLS], F8, tag="pk8")
        nc.sync.dma_start(pk8[:], io["pk8"][:])

        def c32(name):
            o, w = _PK32[name]
            return pk32[:, o:o + w]

        def cr(name):
            o, w = _PKR[name]
            return pkr[:, o:o + w]

        def cb(name):
            o, w = _PKB[name]
            return pkb[:, o:o + w]

        hv_t = cpool.tile([128, npc], F32R, tag="hv_t")
        nc.sync.dma_start(hv_t[:], io["hv_t"][:])
        m2 = cpool.tile([128, npc], BF16, tag="m2")
        # warm the Gelu LUT before the pipeline starts
        warm = small.tile([128, 1], F32, tag="warm")
        nc.scalar.activation(warm[:], c32("epsv"), GELU)

        tc.strict_bb_all_engine_barrier()

        # inputs only the DVE / tail consume - loaded after the barrier so
        # the main loop's first matmuls aren't gated on them
        hv_nat = cpool.tile([128, nblk * 128], F32, tag="hv_nat")
        nc.gpsimd.dma_start(
            hv_nat[:], io["hv_nat"][:].rearrange("(b p) f -> p b f", p=128))
        mask_nat = cpool.tile([128, nblk * K], F32, tag="mask_nat")
        nc.gpsimd.dma_start(
            mask_nat[:], io["mask_nat"][:].rearrange("(b p) k -> p b k", p=128))
        maskv = cpool.tile([128, nblk], F32, tag="maskv")
        nc.gpsimd.dma_start(maskv[:], io["maskv_nat"][:])
        s_mask = cpool.tile([128, nblk], F32, tag="s_mask")
        nc.vector.tensor_reduce(
            s_mask[:], mask_nat[:].rearrange("p (b k) -> p b k", k=K),
            AX.X, ALU.add)

        # mask pre-replicated on host; SWDGE queue so it interleaves with
        # the he stream on the sync queue at packet granularity
        mask_rep = cpool.tile([128, rows], BF16, tag="mask_rep")
        for q in range(8):
            w = rows // 8
            nc.gpsimd.dma_start(mask_rep[:, q * w:(q + 1) * w],
                                io["mask_rep"][:, q * w:(q + 1) * w])

        # hv1 = W1V^T @ h_V, computed once, rounded to bf16
        hv1b = cpool.tile([128, npc], BF16, tag="hv1b")
        with tc.tile_pool(name="pp0", bufs=1, space="PSUM") as pp0:
            ps_hv = pp0.tile([128, npc], F32, tag="pp0")
            nc.tensor.matmul(ps_hv[:], cr("w1vt"), hv_t[:],
                             start=True, stop=True)
            nc.scalar.activation(hv1b[:], ps_hv[:], AF.Copy)

        # tail state (filled per 128-node block as its m2 completes)
        pp = ctx.enter_context(tc.tile_pool(name="pp", bufs=2, space="PSUM"))
        x1n = cpool.tile([128, nblk * 128], F32, tag="x1n")
        h_nat = cpool.tile([128, nblk * 128], F32, tag="h_nat")
        ht2 = cpool.tile([128, npc], F32R, tag="ht2")
        ffr = cpool.tile([128, FCHUNK * npc], F32R, tag="ffr")
        x2n = cpool.tile([128, nblk * 128], F32, tag="x2n")
        out_sb = cpool.tile([128, nblk * 128], F32, tag="out_sb")

        def quake_rstd(dst, v):
            # dst = 1/sqrt(v) via bit hack + 2 Newton iterations (fp32)
            n = v.shape[-1]
            vb = v.bitcast(U32)
            q_t = small.tile([128, nblk], U32, tag="qk_u")
            q = q_t[:, 0:n]
            nc.vector.tensor_scalar(q, vb, 1, 0xFFFFFFFF,
                                    ALU.logical_shift_right, ALU.bitwise_xor)
            # (~h) - (~MAGIC) == MAGIC - h, with no u32 wrap for finite v>0
            nc.vector.tensor_scalar(q, q, 0xFFFFFFFF - MAGIC, None,
                                    ALU.subtract)
            y = dst
            nc.vector.tensor_copy(y, q.bitcast(F32))
            tt_t = small.tile([128, nblk], F32, tag="qk_t")
            t_ = tt_t[:, 0:n]
            for _ in range(1):
                nc.vector.tensor_tensor(t_, y, y, ALU.mult)
                nc.vector.tensor_tensor(t_, t_, v, ALU.mult)
                nc.vector.tensor_scalar(t_, t_, -0.5, 1.5, ALU.mult, ALU.add)
                nc.vector.tensor_tensor(y, y, t_, ALU.mult)

        def layer_norm_blk(dst, x, grep, brep, pfx, j):
            # x: [128, 128] f32 (node-major); LN over the 128-feat axis
            mu_t = small.tile([128, nblk], F32, tag=pfx + "mu")
            mu = mu_t[:, j:j + 1]
            nc.vector.tensor_reduce(mu, x.unsqueeze(1), AX.X, ALU.add)
            nc.vector.tensor_scalar(mu, mu, 1.0 / 128.0, None, ALU.mult)
            xc = lnpool.tile([128, 128], F32, tag="xc")
            nc.vector.tensor_tensor(
                xc[:], x, mu.broadcast_to([128, 128]), ALU.subtract)
            sq = lnpool.tile([128, 128], F32, tag="sq")
            var_t = small.tile([128, nblk], F32, tag=pfx + "var")
            var = var_t[:, j:j + 1]
            nc.vector.tensor_tensor(sq[:], xc[:], xc[:], ALU.mult)
            nc.vector.tensor_reduce(var, sq[:].unsqueeze(1), AX.X, ALU.add)
            nc.vector.tensor_scalar(var, var, 1.0 / 128.0, EPS,
                                    ALU.mult, ALU.add)
            rstd_t = small.tile([128, nblk], F32, tag=pfx + "rstd")
            rstd = rstd_t[:, j:j + 1]
            quake_rstd(rstd, var)
            nc.vector.tensor_tensor(
                xc[:], xc[:], rstd.broadcast_to([128, 128]), ALU.mult)
            nc.vector.tensor_tensor(dst, xc[:], grep, ALU.mult)
            nc.vector.tensor_tensor(dst, dst, brep, ALU.add)

        def tail_block(j):
            # dh^T: stationary = m2 block, moving = W3^T/30 -> [node, feat]
            pd = pp.tile([128, 512], F32, tag="pp")
            nc.tensor.matmul(pd[:, 0:128], m2[:, j * 128:(j + 1) * 128],
                             cb("w3t"), start=True, stop=True)
            xs = x1n[:, j * 128:(j + 1) * 128]
            # xs = (b3rep * s_mask[j]) + pd ; then += hv_nat
            nc.vector.scalar_tensor_tensor(
                xs, c32("b3rep"), s_mask[:, j:j + 1], pd[:, 0:128],
                ALU.mult, ALU.add)
            nc.vector.tensor_tensor(xs, xs,
                                    hv_nat[:, j * 128:(j + 1) * 128], ALU.add)
            hs = h_nat[:, j * 128:(j + 1) * 128]
            layer_norm_blk(hs, xs, c32("g1rep"), c32("b1rep"), "ln1", j)
            pt = pp.tile([128, 512], F32, tag="pp")
            nc.tensor.transpose(pt[:, 0:128], hs, c32("identf"))
            nc.vector.tensor_copy(ht2[:, j * 128:(j + 1) * 128],
                                  pt[:, 0:128])
            # FFN for this block
            for jo in range(FCHUNK):
                pf = pp.tile([128, 512], F32, tag="pp")
                nc.tensor.matmul(pf[:, 0:128],
                                 cr("wint")[:, jo * 128:(jo + 1) * 128],
                                 ht2[:, j * 128:(j + 1) * 128],
                                 start=True, stop=True)
                nc.scalar.activation(
                    ffr[:, jo * npc + j * 128:jo * npc + (j + 1) * 128],
                    pf[:, 0:128], GELU, bias=c32("binp")[:, jo:jo + 1])
            # dh2^T: stationary = ffr chunk block, moving = Wout^T chunk
            pd2 = pp.tile([128, 512], F32, tag="pp")
            for jf in range(FCHUNK):
                nc.tensor.matmul(
                    pd2[:, 0:128],
                    ffr[:, jf * npc + j * 128:jf * npc + (j + 1) * 128],
                    cr("woutt")[:, jf * 128:(jf + 1) * 128],
                    start=(jf == 0), stop=(jf == FCHUNK - 1))
            xs2 = x2n[:, j * 128:(j + 1) * 128]
            nc.vector.scalar_tensor_tensor(
                xs2, c32("boutrep"), 1.0, pd2[:, 0:128], ALU.mult, ALU.add)
            nc.vector.tensor_tensor(xs2, xs2, hs, ALU.add)
            os = out_sb[:, j * 128:(j + 1) * 128]
            layer_norm_blk(os, xs2, c32("g2rep"), c32("b2rep"), "ln2", j)
            nc.vector.tensor_scalar(os, os, maskv[:, j:j + 1], None, ALU.mult)
            nc.sync.dma_start(
                io["out"][:].rearrange("(b p) f -> p b f", p=128)[:, j, :], os)

        # ---- main loop over the h_E stream (W2 lagged one group) ----
        h_et = h_et_d          # [ngrp//4, 128, 4*NCHUNK*GR] fp8
        nodes_g = GR // K      # 16
        with tc.tile_pool(name="p1", bufs=2, space="PSUM") as p1, \
                tc.tile_pool(name="p2", bufs=1, space="PSUM") as p2:

            he4s = {0: he4_0}

            def w1_stage(g):
                q, g4 = divmod(g, 4)
                if g4 == 0 and q not in he4s:
                    he4 = hpool.tile([128, 4, NCHUNK * GR], F8, tag="he")
                    nc.sync.dma_start(he4[:], h_et[q])
                    he4s.clear()
                    he4s[q] = he4
                he = he4s[q][:, g4, :]
                ps1 = p1.tile([128, QT * 512], F32, tag="ps1")
                for t in range(QT):
                    tn = g * QT * NPT + t * NPT
                    o = 512 * t
                    hv_rep = hv1b[:, tn:tn + NPT].unsqueeze(2) \
                        .broadcast_to([128, NPT, K])
                    nc.tensor.matmul(ps1[:, o:o + TT], cb("identb"),
                                     hv_rep, start=True, stop=False)
                    for c in range(NCHUNK):
                        nc.tensor.matmul(
                            ps1[:, o:o + TT],
                            pk8[:, c * 128:(c + 1) * 128],
                            he[:, c * GR + t * TT:c * GR + (t + 1) * TT],
                            start=False, stop=(c == NCHUNK - 1))
                return ps1

            def gelu1_stage(g, ps1):
                g1 = wpool.tile([128, GR], BF16, tag="g1")
                ps1v = ps1[:].rearrange("p (h c) -> p h c", h=QT)
                nc.scalar.activation(
                    g1[:].rearrange("p (h c) -> p h c", h=QT),
                    ps1v[:, :, 0:TT], GELU, bias=c32("b1"))
                return g1

            def w2_stage(g1):
                ps2 = p2.tile([128, QT * 512], F32, tag="ps2")
                nc.tensor.matmul(ps2[:, 0:512], cb("w2t"), g1[:, 0:512],
                                 start=True, stop=True)
                nc.tensor.matmul(ps2[:, 512:768], cb("w2t"), g1[:, 512:768],
                                 start=True, stop=True)
                return ps2

            h2pairs = {}

            def rest_stage(g, ps2):
                # gelu2 into half of a 2-group tile; mask+tree per pair
                if g % 2 == 0:
                    h2p = wpool.tile([128, 2 * GR], BF16, tag="h2")
                    h2pairs[0] = h2p
                h2 = h2pairs[0]
                nc.scalar.activation(h2[:, (g % 2) * GR:(g % 2 + 1) * GR],
                                     ps2[:, 0:GR], GELU, bias=c32("b2"))
                if g % 2 == 0:
                    return
                r0 = (g - 1) * GR
                h2m = wpool.tile([128, 2 * GR], BF16, tag="h2m")
                nc.vector.tensor_tensor(
                    h2m[:], h2[:], mask_rep[:, r0:r0 + 2 * GR], ALU.mult)
                with nc.allow_low_precision(
                        reason="k-sum tree in bf16; dh is small next to the "
                               "residual stream"):
                    h2mv = h2m[:].rearrange("p (n k) -> p n k", k=K)
                    ng2 = 2 * nodes_g
                    t24 = wpool.tile([128, ng2, 24], BF16, tag="t24")
                    nc.vector.tensor_tensor(
                        t24[:], h2mv[:, :, 0:24], h2mv[:, :, 24:48], ALU.add)
                    t12 = wpool.tile([128, ng2, 12], BF16, tag="t12")
                    nc.vector.tensor_tensor(
                        t12[:], t24[:, :, 0:12], t24[:, :, 12:24], ALU.add)
                    nc.vector.tensor_reduce(
                        m2[:, (g - 1) * nodes_g:(g + 1) * nodes_g], t12[:],
                        AX.X, ALU.add)

            prev = None  # (g, ps1)
            lag2 = None  # (g, g1)
            for g in range(ngrp):
                ps1 = w1_stage(g)
                if lag2 is not None:
                    gl, g1l = lag2
                    rest_stage(gl, w2_stage(g1l))
                    lag2 = None
                    if (gl + 1) % gpb == 0:
                        tail_block(gl // gpb)
                if prev is not None:
                    gp, ps1p = prev
                    lag2 = (gp, gelu1_stage(gp, ps1p))
                prev = (g, ps1)
            # drain
            gp, ps1p = prev
            if lag2 is not None:
                gl, g1l = lag2
                rest_stage(gl, w2_stage(g1l))
                if (gl + 1) % gpb == 0:
                    tail_block(gl // gpb)
            g1 = gelu1_stage(gp, ps1p)
            rest_stage(gp, w2_stage(g1))
            tail_block(nblk - 1)


def build_nc(npc):
    rows = npc * K
    nblk = npc // 128
    nc = bacc.Bacc()
    io = {}

    def inp(name, shape, dt=F32):
        io[name] = nc.dram_tensor(name, shape, dt, kind="ExternalInput")

    inp("h_et", [rows // GR // 4, 128, 4 * NCHUNK * GR], F8)
    inp("hv_t", [128, npc], F32R)
    inp("hv_nat", [npc, H])
    inp("mask_rep", [128, rows], BF16)
    inp("mask_nat", [npc, K])
    inp("maskv_nat", [128, nblk])
    inp("pk32", [128, PK32_COLS])
    inp("pkr", [128, PKR_COLS], F32R)
    inp("pkb", [128, PKB_COLS], BF16)
    inp("pk8", [128, PK8_COLS], F8)
    io["out"] = nc.dram_tensor("out", [npc, H], F32, kind="ExternalOutput")
    _emit(nc, io, npc)
    return nc


def prep_maps(h_V, h_E, mask_V, mask_attend,
              W1_w, W1_b, W2_w, W2_b, W3_w, W3_b,
              ln1_g, ln1_b, ln2_g, ln2_b,
              Win_w, Win_b, Wout_w, Wout_b, ncores):
    import ml_dtypes
    f32 = np.float32
    bf16 = ml_dtypes.bfloat16
    f8 = ml_dtypes.float8_e4m3
    B, N, Kk, _ = h_E.shape
    nodes = B * N
    npc = nodes // ncores
    rows = npc * Kk
    nblk = npc // 128

    GRl = TT * QT
    ngrp = rows // GRl
    hE = np.asarray(h_E, f32).reshape(ncores, rows, NIN).astype(f8)
    # [core, chunk, 128, q, g4, GR] -> [core, q, 128, g4, chunk, GR]
    h_et = np.ascontiguousarray(
        hE.transpose(0, 2, 1).reshape(ncores, NCHUNK, 128, ngrp // 4, 4, GRl)
        .transpose(0, 3, 2, 4, 1, 5)).reshape(
        ncores, ngrp // 4, 128, 4 * NCHUNK * GRl)

    hv = np.asarray(h_V, f32).reshape(ncores, npc, H)
    hv_t = np.ascontiguousarray(hv.transpose(0, 2, 1))
    mA = np.asarray(mask_attend, f32).reshape(ncores, npc, Kk)
    mV = np.asarray(mask_V, f32).reshape(ncores, nblk, 128)
    maskv_nat = np.ascontiguousarray(mV.transpose(0, 2, 1))

    def t(x):
        return np.asarray(x, f32).T

    rep = lambda v: np.tile(np.asarray(v, f32).reshape(1, -1), (128, 1))

    pk32 = np.zeros((128, PK32_COLS), f32)

    def put32(name, arr):
        o, w = _PK32[name]
        pk32[:, o:o + w] = arr

    put32("b1", np.asarray(W1_b, f32).reshape(128, 1))
    put32("b2", np.asarray(W2_b, f32).reshape(128, 1))
    put32("b3rep", rep(np.asarray(W3_b, f32) / SCALE))
    put32("binp", np.asarray(Win_b, f32).reshape(FCHUNK, 128).T)
    put32("boutrep", rep(Wout_b))
    put32("g1rep", rep(ln1_g))
    put32("b1rep", rep(ln1_b))
    put32("g2rep", rep(ln2_g))
    put32("b2rep", rep(ln2_b))
    put32("identf", np.eye(128, dtype=f32))
    put32("epsv", np.full((128, 1), EPS, f32))

    pkr = np.zeros((128, PKR_COLS), f32)
    pkr[:, 0:128] = t(np.asarray(W1_w, f32)[:, :H])
    pkr[:, 128:640] = t(Win_w)
    pkr[:, 640:1152] = np.asarray(Wout_w, f32).T.reshape(
        FCHUNK, 128, 128).transpose(1, 0, 2).reshape(128, 512)

    pkb = np.zeros((128, PKB_COLS), f32)
    pkb[:, 0:128] = t(W2_w)
    pkb[:, 128:256] = t(np.asarray(W3_w, f32) / SCALE)
    pkb[:, 256:384] = np.eye(128, dtype=f32)

    # W1E fp8 chunk stationaries: pk8[p, c*128+m] = W1E[m, c*128+p]
    W1E = np.asarray(W1_w, f32)[:, H:].T.astype(f8)  # [384 in, 128 out]
    pk8 = np.ascontiguousarray(
        W1E.reshape(NCHUNK, 128, 128).transpose(1, 0, 2).reshape(128, -1))

    shared = {
        "pk32": pk32,
        "pkr": pkr,
        "pkb": pkb.astype(bf16),
        "pk8": pk8,
    }
    in_maps = []
    for c in range(ncores):
        m = dict(shared)
        m["h_et"] = h_et[c]
        m["hv_t"] = hv_t[c]
        m["hv_nat"] = np.ascontiguousarray(hv[c])
        m["mask_rep"] = np.ascontiguousarray(np.broadcast_to(
            mA[c].reshape(1, rows), (128, rows))).astype(bf16)
        m["mask_nat"] = np.ascontiguousarray(mA[c])
        m["maskv_nat"] = maskv_nat[c]
        in_maps.append(m)
    return in_maps, npc


_NC_CACHE = {}


def _get_nc(npc):
    if npc not in _NC_CACHE:
        nc = build_nc(npc)
        nc.finalize()
        _NC_CACHE[npc] = nc
    return _NC_CACHE[npc]


def run(inputs, trace=False):
    B, N, _, _ = inputs["h_E"].shape
    in_maps, npc = prep_maps(ncores=NCORES, **inputs)
    nc = _get_nc(npc)
    res = run_bass_kernel_spmd(nc, in_maps, core_ids=list(range(NCORES)),
                               trace=trace)
    out = np.concatenate([res.results[c]["out"] for c in range(NCORES)],
                         axis=0).reshape(B, N, H).astype(np.float32)
    return out, res.exec_time_ns


def kernel(**inputs) -> np.ndarray:
    out, _ = run(inputs)
    return out
